# revision 23
# baseline (speedup 1.0000x reference)
"""MinibatchDiscrimination Bass kernel for 8 TRN2 NeuronCores.

out[i,o] = sum_{j!=i} exp(-sum_k |M[i,k,o]-M[j,k,o]|),  M = x @ T.

Strategy: cyclic-offset pairing over the symmetric BxB distance matrix.
Shift t pairs row i with row (i+t) mod B; t=1..B/2 covers every unordered
pair exactly once (t=B/2 twice -> halved via an exp bias of -ln2). Core c
computes shifts t in [16c+1, 16c+16]; each pair contributes to both rows.
Host sums the 8 partial outputs with the reference's fp32 `(1+s)-1`
absorption.

Key identity (one DVE pass per shift):
    sum_k |a_k - b_k| = 2*sum_k max(a_k, b_k) - sum_k a_k - sum_k b_k
The k-reduction of the max tensor runs on the PE as column-tiled matmul
pairs (two 64-wide selection matmuls concurrent in separate array column
halves: even half over chunks 0..18, odd half over 19..37), then one fold
matmul adds the two PSUM halves. Row-sum corrections use a host-precomputed
k-summed T (sa = x @ TS), removing the per-chunk row-sum matmuls entirely.

Schedule notes (from perfetto analysis of the 146us baseline):
 - the DVE max pass is the hard floor (~5.2us/shift, fp16 2x mode); all
   other engines are kept off its SBUF port (GPSIMD idle: it shares the
   DVE port and was degrading TTs to 1x),
 - max instrs are merged to one per chunk-half (2/shift instead of 4),
 - h0 max passes for the first shifts are emitted interleaved into the
   phase-1 loop so the DVE streams during the tail of the M matmul,
 - the i-doubling copies run on DVE (SBUF 4x), the chunk copies on ACT,
   the odd-shift alignment copies (mtb) on ACT spread over early shifts.
"""

import numpy as np

B = 256
F = 512
K = 75
O = 64
KO = K * O          # 4800
KOP = 4864          # padded to 38*128
NCH = KOP // 128    # 38 ko-chunks
NH = NCH // 2       # 19 chunks per half
CWA = 384           # chunk width: M^T[.., i] doubled to i in [0,384)
NSLOT = 16
LN2 = float(np.log(2.0))
DSB_SCALE = 1.0 / 16.0   # fp16 pre-scale of the max-sum before the fold
START_HI = True          # PSUM start=True clears are partition-scoped
                         # (measured): each column-half group needs its own

_NC_CACHE = {}


def _build_nc():
    import concourse.bacc as bacc
    import concourse.bass as bass
    import concourse.mybir as mybir
    from concourse import tile

    fp16 = mybir.dt.float16
    fp32 = mybir.dt.float32
    bf16 = mybir.dt.bfloat16
    i32 = mybir.dt.int32
    Alu = mybir.AluOpType
    Act = mybir.ActivationFunctionType

    nc = bacc.Bacc(
        "TRN2", target_bir_lowering=False, debug=False, num_devices=8
    )

    with tile.TileContext(nc) as tc:
        xt_d = nc.dram_tensor("xt", [128, 1024], fp16, kind="ExternalInput")
        tt_d = nc.dram_tensor("tt", [128, NCH * 512], fp16, kind="ExternalInput")
        ts_d = nc.dram_tensor("tsum", [128, 256], fp16, kind="ExternalInput")
        ss_d = nc.dram_tensor("ssel", [128, 64], fp16, kind="ExternalInput")
        nh_d = nc.dram_tensor("nhalf", [64, 64], fp16, kind="ExternalInput")
        id_d = nc.dram_tensor("ident", [64, 64], bf16, kind="ExternalInput")
        of_d = nc.dram_tensor("offs", [1, 2], i32, kind="ExternalInput")
        bi_d = nc.dram_tensor("bias", [64, NSLOT], fp32, kind="ExternalInput")
        out_d = nc.dram_tensor("out", [64, 768], fp32, kind="ExternalOutput")

        with (
            tc.tile_pool(name="const", bufs=1) as cpool,
            tc.tile_pool(name="tload", bufs=3) as tpool,
            tc.tile_pool(name="mxh0", bufs=7) as xpool0,
            tc.tile_pool(name="mxh1", bufs=3) as xpool1,
            tc.tile_pool(name="dsb", bufs=2) as spool,
            tc.tile_pool(name="esb", bufs=3) as epool,
            tc.tile_pool(name="mpsum", bufs=2, space="PSUM") as mpsum,
            tc.tile_pool(name="dpsum", bufs=2, space="PSUM") as dpsum,
            tc.tile_pool(name="d2psum", bufs=2, space="PSUM") as d2psum,
            tc.tile_pool(name="apsum", bufs=1, space="PSUM") as apsum,
        ):
            # ss first (it feeds the PE warm-up burst), then the first T
            # chunk-pair and xt so the first real matmul starts early
            ss = cpool.tile([128, 64], fp16)
            nc.sync.dma_start(ss[:, :], ss_d[:, :])
            tsb0 = tpool.tile([128, 1024], fp16, tag="tsb")
            nc.sync.dma_start(tsb0[:, :], tt_d[:, 0:1024])
            xt = cpool.tile([128, 1024], fp16)
            nc.sync.dma_start(xt[:, :], xt_d[:, :])
            tsum = cpool.tile([128, 256], fp16)
            nc.sync.dma_start(tsum[:, :], ts_d[:, :])
            offs = cpool.tile([1, 2], i32)
            nc.sync.dma_start(offs[:, :], of_d[:, :])
            nh = cpool.tile([64, 64], fp16)
            nc.sync.dma_start(nh[:, :], nh_d[:, :])
            ident = cpool.tile([64, 64], bf16)
            nc.sync.dma_start(ident[:, :], id_d[:, :])
            bias = cpool.tile([64, NSLOT], fp32)
            nc.sync.dma_start(bias[:, :], bi_d[:, :])

            # chunk-half tiles: [0]=chunks 0..18, [1]=chunks 19..37
            mta = [
                cpool.tile([128, NH * CWA], fp16, name=f"mta{h}", tag=f"mta{h}")
                for h in (0, 1)
            ]
            mtb = [
                cpool.tile([128, NH * CWA], fp16, name=f"mtb{h}", tag=f"mtb{h}")
                for h in (0, 1)
            ]
            mta3 = [t[:, :].rearrange("p (c w) -> p c w", w=CWA) for t in mta]
            mtb3 = [t[:, :].rearrange("p (c w) -> p c w", w=CWA) for t in mtb]
            sa2 = cpool.tile([64, 512], fp16)

            ps_self = apsum.tile([64, 256], fp32, tag="pself")
            ps_pair = apsum.tile([64, 512], fp32, tag="ppair")
            nc.vector.memset(ps_pair[:, :], 0.0)

            # one register load of t0 = 16*core + 1 per engine (the PE one
            # is emitted after phase 1 so it never delays the warm-up)
            rtv = nc.vector.alloc_register("t0v")
            nc.vector.reg_load(rtv, offs[0:1, 0:1])
            vt0 = nc.vector.snap(rtv, donate=True, min_val=1, max_val=113)

            # HAM warm-up: ~3.6us of tiny matmuls into a scratch psum bank
            # while the tsb0/xt DMAs stream, so the activity monitor lifts
            # the PE clock gate to 2.4GHz before the real matmuls begin.
            warm = mpsum.tile([128, 512], fp32, tag="mp")
            for i in range(40):
                nc.tensor.matmul(
                    warm[0:64, 0:64],
                    ss[:, 0:64],
                    ss[:, 0:64],
                    start=(i == 0),
                    stop=(i == 39),
                )

            # Slot order: even-t slots (odd s) first - they need only mta;
            # odd-t slots run last, after the mtb shift-copies are built.
            order = [s for s in range(NSLOT) if s % 2 == 1] + [
                s for s in range(NSLOT) if s % 2 == 0
            ]

            # h0 max prefetch: slot order[0] is staircased through phase-1
            # chunk production (sub-range max passes as chunks land), the
            # next slots get full h0 passes spread over later kops so each
            # interleaved DVE doubling is data-ready in FIFO order.
            stair = {2: (0, 6), 4: (6, 10), 6: (10, 14), 8: (14, 18), 9: (18, 19)}
            pref_after = {9: (1,), 11: (2,), 13: (3,), 15: (4,), 17: (5,)}
            mx_pre = {}

            def emit_tt(si, h, c0=0, c1=NH, m3=None):
                """Emit the max pass for slot order[si], chunk-half h,
                chunk range [c0, c1). Returns the mx tile (3d view)."""
                s = order[si]
                par = (s + 1) % 2  # t parity; even t -> mta, odd t -> mtb
                if m3 is None:
                    pool = xpool0 if h == 0 else xpool1
                    mx = pool.tile([128, NH * 256], fp16, tag=f"mx{h}")
                    m3 = mx[:, :].rearrange("p (c w) -> p c w", w=256)
                src3 = mta3 if par == 0 else mtb3
                off = vt0 + (s - par)
                nc.vector.tensor_tensor(
                    m3[:, c0:c1, :],
                    mta3[h][:, c0:c1, 0:256],
                    src3[h][:, c0:c1, bass.ds(off, 256)],
                    Alu.max,
                )
                return m3

            # Phase 1: MT = M^T in (ko-chunk, i) layout, i doubled to 384.
            # Per kop (2 ko chunks): 8 matmuls -> mp psum, ACT copies the
            # 256-wide blocks out, DVE duplicates i 0..127 to 256..384.
            for kop in range(NCH // 2):
                ko0 = 2 * kop
                if kop == 0:
                    tsb = tsb0
                else:
                    tsb = tpool.tile([128, 1024], fp16, tag="tsb")
                    nc.sync.dma_start(
                        tsb[:, :], tt_d[:, ko0 * 512 : (ko0 + 2) * 512]
                    )
                mp = mpsum.tile([128, 512], fp32)
                for k2 in range(2):
                    for cc in range(4):
                        nc.tensor.matmul(
                            mp[:, k2 * 256 : (k2 + 1) * 256],
                            tsb[:, (k2 * 4 + cc) * 128 : (k2 * 4 + cc + 1) * 128],
                            xt[:, cc * 256 : (cc + 1) * 256],
                            start=(cc == 0),
                            stop=(cc == 3),
                        )
                mp3 = mp[:, :].rearrange("p (k w) -> p k w", k=2)
                h0, kh0 = divmod(ko0, NH)
                h1, kh1 = divmod(ko0 + 1, NH)
                ui = mybir.dt.uint32
                if h0 == h1:
                    # one strided copy for both chunks, one int32 doubling
                    nc.scalar.copy(
                        mta3[h0][:, kh0 : kh0 + 2, 0:256], mp3[:, :, :]
                    )
                    nc.vector.tensor_copy(
                        mta3[h0][:, kh0 : kh0 + 2, 256:384].bitcast(ui),
                        mta3[h0][:, kh0 : kh0 + 2, 0:128].bitcast(ui),
                    )
                else:  # kop 9 spans the two half tiles
                    for k2, (h, kh) in enumerate(((h0, kh0), (h1, kh1))):
                        nc.scalar.copy(
                            mta3[h][:, kh : kh + 1, 0:256], mp3[:, k2 : k2 + 1, :]
                        )
                        nc.vector.tensor_copy(
                            mta3[h][:, kh : kh + 1, 256:384].bitcast(ui),
                            mta3[h][:, kh : kh + 1, 0:128].bitcast(ui),
                        )
                if kop == 2:
                    # row-sum corrections from host-precomputed k-summed T:
                    # sa[o, i] = sum_k M[i, k, o] = (TS.T @ x.T)[o, i]
                    sa_ps = d2psum.tile([64, 256], fp32, tag="d2")
                    for cc in range(4):
                        nc.tensor.matmul(
                            sa_ps[:, :],
                            tsum[:, cc * 64 : (cc + 1) * 64],
                            xt[:, cc * 256 : (cc + 1) * 256],
                            start=(cc == 0),
                            stop=(cc == 3),
                        )
                    nc.scalar.copy(sa2[:, 0:256], sa_ps[:, :])
                    nc.scalar.copy(sa2[:, 256:512], sa_ps[:, :])
                if kop in stair:
                    c0, c1 = stair[kop]
                    mx_pre[0] = emit_tt(0, 0, c0, c1, mx_pre.get(0))
                if kop in pref_after:
                    for si in pref_after[kop]:
                        mx_pre[si] = emit_tt(si, 0)

            rtp = nc.tensor.alloc_register("t0p")
            nc.tensor.reg_load(rtp, offs[0:1, 0:1])
            vp0 = nc.tensor.snap(rtp, donate=True, min_val=1, max_val=113)

            # odd-shift alignment copies, spread over the first 8 slots on
            # ACT: mtb[.., j] = mta[.., j+1]
            mtb_jobs = [(h, kh) for h in (0, 1) for kh in range(NH)]

            def emit_mtb(n):
                while n > 0 and mtb_jobs:
                    h, kh = mtb_jobs.pop(0)
                    ba = kh * CWA
                    nc.scalar.copy(
                        mtb[h][:, ba : ba + 383], mta[h][:, ba + 1 : ba + 384]
                    )
                    n -= 1

            # Phase 2, per shift slot s (t = t0 + s):
            #   2 DVE max instrs (fp16 2x, one per chunk-half)
            #   -> PE: 19 column-tiled matmul pairs (even half -> psum rows
            #   0:64, odd half -> 64:128), ACT copies /16 to fp16, PE fold
            #   matmul adds the halves + 2 row-sum corrections -> ACT
            #   exp(scale=-32, bias) -> PE accumulates e into self/pair.
            # Staggered PE pipeline: per slot, the queue gets
            #   lo-run(s) | fold+corr(s-1) | acc(s-2) | hi-run(s)
            # so no matmul ever heads the queue waiting on an ACT result
            # (dsb of s-1 and exp of s-2 are long done by the time their
            # consumers are reached), and the lo/hi runs on opposite array
            # column halves stay adjacent enough to overlap.
            def emit_fc(si):
                """fold + corrections for slot order[si] -> dp2, exp -> e."""
                s = order[si]
                dsb = state[si]["dsb"]
                dp2 = d2psum.tile([64, 256], fp32, tag="d2")
                nc.tensor.matmul(
                    dp2[:, :], ss[:, 0:64], dsb[:, :], start=True, stop=False
                )
                nc.tensor.matmul(
                    dp2[:, :], nh[:, :], sa2[:, 0:256], start=False, stop=False
                )
                nc.tensor.matmul(
                    dp2[:, :],
                    nh[:, :],
                    sa2[:, bass.ds(vp0 + s, 256)],
                    start=False,
                    stop=True,
                )
                e = epool.tile([64, 256], bf16, tag="e")
                nc.scalar.activation(
                    e[:, :],
                    dp2[:, :],
                    Act.Exp,
                    bias=bias[:, s : s + 1],
                    scale=-2.0 / DSB_SCALE,
                )
                state[si]["e"] = e

            def emit_ac(si):
                """accumulate e of slot order[si] into self/pair psums."""
                s = order[si]
                e = state[si]["e"]
                nc.tensor.matmul(
                    ps_self[:, :],
                    ident[:, :],
                    e[:, :],
                    start=(si == 0),
                    stop=(si == NSLOT - 1),
                )
                nc.tensor.matmul(
                    ps_pair[:, bass.ds(vp0 + s, 256)],
                    ident[:, :],
                    e[:, :],
                    start=False,
                    stop=(si == NSLOT - 1),
                    skip_group_check=True,
                )

            state = {}
            for si, s in enumerate(order):
                if si in mx_pre:
                    m3a = mx_pre.pop(si)
                else:
                    m3a = emit_tt(si, 0)
                if si == NSLOT - 1:
                    # split the last h1 max so its hi matmuls start earlier
                    m3b = emit_tt(si, 1, 0, 10)
                    emit_tt(si, 1, 10, NH, m3b)
                else:
                    m3b = emit_tt(si, 1)
                if si >= 1:
                    emit_fc(si - 1)
                if si >= 2:
                    emit_ac(si - 2)
                dp = dpsum.tile([128, 256], fp32, tag="dp")
                # interleaved column-tiled pairs: lo/hi matmuls adjacent in
                # the queue run concurrently in opposite array halves
                for c in range(NH):
                    nc.tensor.matmul(
                        dp[0:64, :],
                        ss[:, 0:64],
                        m3a[:, c, :],
                        start=(c == 0),
                        stop=(c == NH - 1),
                        tile_position=(0, 0),
                    )
                    nc.tensor.matmul(
                        dp[64:128, :],
                        ss[:, 0:64],
                        m3b[:, c, :],
                        start=(c == 0 and START_HI),
                        stop=(c == NH - 1),
                        tile_position=(0, 64),
                        skip_group_check=True,
                    )
                dsb = spool.tile([128, 256], fp16, tag="dsb")
                nc.scalar.mul(dsb[:, :], dp[:, :], DSB_SCALE)
                state[si] = {"dsb": dsb}
                if si < 8:
                    emit_mtb(5)
            emit_fc(NSLOT - 1)
            emit_ac(NSLOT - 2)
            emit_ac(NSLOT - 1)

            # raw accumulators out; the host combine (which already sums
            # the 8 cores) folds pair[0:256]+pair[256:512]+self
            outsb = cpool.tile([64, 768], fp32)
            nc.scalar.copy(outsb[:, 0:512], ps_pair[:, :])
            nc.scalar.copy(outsb[:, 512:768], ps_self[:, :])
            nc.sync.dma_start(out_d[:, :], outsb[:, :])

    nc.compile()
    return nc


def get_nc():
    if "nc" not in _NC_CACHE:
        _NC_CACHE["nc"] = _build_nc()
    return _NC_CACHE["nc"]


def host_inputs(x, T):
    """Host-side shard prep: returns the 8 per-core input maps."""
    x = np.asarray(x, dtype=np.float32)
    T = np.asarray(T, dtype=np.float32)
    T2p = np.zeros((F, KOP), np.float32)
    T2p[:, :KO] = T.reshape(F, KO)
    # tt[p, ko*512 + cc*128 + j] = T2p[cc*128+p, ko*128+j]
    tt = (
        np.ascontiguousarray(
            T2p.reshape(4, 128, NCH, 128).transpose(1, 2, 0, 3)
        )
        .reshape(128, NCH * 512)
        .astype(np.float16)
    )
    # xt[p, cc*256 + i] = x[i, cc*128+p]
    xt = (
        np.ascontiguousarray(x.T.reshape(4, 128, B).transpose(1, 0, 2))
        .reshape(128, 1024)
        .astype(np.float16)
    )
    # tsum[p, cc*64 + o] = TS[cc*128+p, o],  TS[f, o] = sum_k T[f, k, o]
    TS = T.reshape(F, K, O).sum(axis=1)
    tsum = (
        np.ascontiguousarray(TS.reshape(4, 128, O).transpose(1, 0, 2))
        .reshape(128, 256)
        .astype(np.float16)
    )
    ss = (np.arange(128)[:, None] % 64 == np.arange(64)[None, :]).astype(
        np.float16
    )
    import ml_dtypes
    nh = (-0.5 * DSB_SCALE * np.eye(64)).astype(np.float16)
    ident = np.eye(64).astype(ml_dtypes.bfloat16)
    in_maps = []
    for c in range(8):
        offs = np.array([[16 * c + 1, 0]], np.int32)
        biases = np.zeros((64, NSLOT), np.float32)
        if c == 7:
            biases[:, 15] = -LN2  # t = 128: every pair covered twice
        in_maps.append(
            {
                "xt": xt,
                "tt": tt,
                "tsum": tsum,
                "ssel": ss,
                "nhalf": nh,
                "ident": ident,
                "offs": offs,
                "bias": biases,
            }
        )
    return in_maps


def combine(results):
    """Sum per-core partial outputs [64,256] -> full [256,64] fp32.

    The reference computes sum_j exp(-d) (including the j=i term, = 1.0) in
    fp32 and then subtracts 1.0. Replicate those fp32 semantics exactly: the
    off-diagonal terms here are ~1e-25 and are fully absorbed by the +1.
    """
    acc = np.zeros((64, 256), np.float64)
    for r in results:
        o = r["out"].astype(np.float64)
        acc += o[:, 0:256] + o[:, 256:512] + o[:, 512:768]
    full = np.ascontiguousarray(acc.T).astype(np.float32)
    return (np.float32(1.0) + full) - np.float32(1.0)


def run_on_hw(x, T, trace=False):
    from concourse.bass_utils import run_bass_kernel_spmd

    nc = get_nc()
    in_maps = host_inputs(x, T)
    res = run_bass_kernel_spmd(
        nc, in_maps, core_ids=list(range(8)), trace=trace
    )
    return combine(res.results), res


def kernel(x, T):
    out, _ = run_on_hw(x, T, trace=False)
    return out


# revision 26
# speedup vs baseline: 1.0139x; 1.0139x over previous
"""MinibatchDiscrimination Bass kernel for 8 TRN2 NeuronCores.

out[i,o] = sum_{j!=i} exp(-sum_k |M[i,k,o]-M[j,k,o]|),  M = x @ T.

Strategy: cyclic-offset pairing over the symmetric BxB distance matrix.
Shift t pairs row i with row (i+t) mod B; t=1..B/2 covers every unordered
pair exactly once (t=B/2 twice -> halved via an exp bias of -ln2). Core c
computes shifts t in [16c+1, 16c+16]; each pair contributes to both rows.
Host sums the 8 partial outputs with the reference's fp32 `(1+s)-1`
absorption.

Key identity (one DVE pass per shift):
    sum_k |a_k - b_k| = 2*sum_k max(a_k, b_k) - sum_k a_k - sum_k b_k
The k-reduction of the max tensor runs on the PE as column-tiled matmul
pairs (two 64-wide selection matmuls concurrent in separate array column
halves: even half over chunks 0..18, odd half over 19..37), then one fold
matmul adds the two PSUM halves. Row-sum corrections use a host-precomputed
k-summed T (sa = x @ TS), removing the per-chunk row-sum matmuls entirely.

Schedule notes (from perfetto analysis of the 146us baseline):
 - the DVE max pass is the hard floor (~5.2us/shift, fp16 2x mode); all
   other engines are kept off its SBUF port (GPSIMD idle: it shares the
   DVE port and was degrading TTs to 1x),
 - max instrs are merged to one per chunk-half (2/shift instead of 4),
 - h0 max passes for the first shifts are emitted interleaved into the
   phase-1 loop so the DVE streams during the tail of the M matmul,
 - the i-doubling copies run on DVE (SBUF 4x), the chunk copies on ACT,
   the odd-shift alignment copies (mtb) on ACT spread over early shifts.
"""

import numpy as np

B = 256
F = 512
K = 75
O = 64
KO = K * O          # 4800
KOP = 4864          # padded to 38*128
NCH = KOP // 128    # 38 ko-chunks
NH = NCH // 2       # 19 chunks per half
CWA = 384           # chunk width: M^T[.., i] doubled to i in [0,384)
NSLOT = 16
LN2 = float(np.log(2.0))
DSB_SCALE = 1.0 / 16.0   # fp16 pre-scale of the max-sum before the fold
START_HI = True          # PSUM start=True clears are partition-scoped
                         # (measured): each column-half group needs its own

_NC_CACHE = {}


def _build_nc():
    import concourse.bacc as bacc
    import concourse.bass as bass
    import concourse.mybir as mybir
    from concourse import tile

    fp16 = mybir.dt.float16
    fp32 = mybir.dt.float32
    bf16 = mybir.dt.bfloat16
    i32 = mybir.dt.int32
    Alu = mybir.AluOpType
    Act = mybir.ActivationFunctionType

    nc = bacc.Bacc(
        "TRN2", target_bir_lowering=False, debug=False, num_devices=8
    )

    with tile.TileContext(nc) as tc:
        xt_d = nc.dram_tensor("xt", [128, 1024], fp16, kind="ExternalInput")
        tt_d = nc.dram_tensor("tt", [128, NCH * 512], fp16, kind="ExternalInput")
        ts_d = nc.dram_tensor("tsum", [128, 256], fp16, kind="ExternalInput")
        ss_d = nc.dram_tensor("ssel", [128, 64], fp16, kind="ExternalInput")
        nh_d = nc.dram_tensor("nhalf", [64, 64], fp16, kind="ExternalInput")
        id_d = nc.dram_tensor("ident", [64, 64], bf16, kind="ExternalInput")
        of_d = nc.dram_tensor("offs", [1, 2], i32, kind="ExternalInput")
        bi_d = nc.dram_tensor("bias", [64, NSLOT], fp32, kind="ExternalInput")
        out_d = nc.dram_tensor("out", [64, 768], fp32, kind="ExternalOutput")

        with (
            tc.tile_pool(name="const", bufs=1) as cpool,
            tc.tile_pool(name="tload", bufs=3) as tpool,
            tc.tile_pool(name="mxh0", bufs=7) as xpool0,
            tc.tile_pool(name="mxh1", bufs=3) as xpool1,
            tc.tile_pool(name="dsb", bufs=2) as spool,
            tc.tile_pool(name="esb", bufs=3) as epool,
            tc.tile_pool(name="mpsum", bufs=2, space="PSUM") as mpsum,
            tc.tile_pool(name="dpsum", bufs=2, space="PSUM") as dpsum,
            tc.tile_pool(name="d2psum", bufs=2, space="PSUM") as d2psum,
            tc.tile_pool(name="apsum", bufs=1, space="PSUM") as apsum,
        ):
            # prefetch the first T chunk-pair first so the first matmul can
            # start as early as possible; xt as one row-contiguous DMA
            tsb0 = tpool.tile([128, 1024], fp16, tag="tsb")
            nc.sync.dma_start(tsb0[:, :], tt_d[:, 0:1024])
            xt = cpool.tile([128, 1024], fp16)
            nc.sync.dma_start(xt[:, :], xt_d[:, :])
            tsum = cpool.tile([128, 256], fp16)
            nc.sync.dma_start(tsum[:, :], ts_d[:, :])
            offs = cpool.tile([1, 2], i32)
            nc.sync.dma_start(offs[:, :], of_d[:, :])
            ss = cpool.tile([128, 64], fp16)
            nc.sync.dma_start(ss[:, :], ss_d[:, :])
            nh = cpool.tile([64, 64], fp16)
            nc.sync.dma_start(nh[:, :], nh_d[:, :])
            ident = cpool.tile([64, 64], bf16)
            nc.sync.dma_start(ident[:, :], id_d[:, :])
            bias = cpool.tile([64, NSLOT], fp32)
            nc.sync.dma_start(bias[:, :], bi_d[:, :])

            # chunk-half tiles: [0]=chunks 0..18, [1]=chunks 19..37
            mta = [
                cpool.tile([128, NH * CWA], fp16, name=f"mta{h}", tag=f"mta{h}")
                for h in (0, 1)
            ]
            mtb = [
                cpool.tile([128, NH * CWA], fp16, name=f"mtb{h}", tag=f"mtb{h}")
                for h in (0, 1)
            ]
            mta3 = [t[:, :].rearrange("p (c w) -> p c w", w=CWA) for t in mta]
            mtb3 = [t[:, :].rearrange("p (c w) -> p c w", w=CWA) for t in mtb]
            sa2 = cpool.tile([64, 512], fp16)

            ps_self = apsum.tile([64, 256], fp32, tag="pself")
            ps_pair = apsum.tile([64, 512], fp32, tag="ppair")
            nc.vector.memset(ps_pair[:, :], 0.0)

            # one register load of t0 = 16*core + 1 per engine
            rtv = nc.vector.alloc_register("t0v")
            nc.vector.reg_load(rtv, offs[0:1, 0:1])
            vt0 = nc.vector.snap(rtv, donate=True, min_val=1, max_val=113)
            rtp = nc.tensor.alloc_register("t0p")
            nc.tensor.reg_load(rtp, offs[0:1, 0:1])
            vp0 = nc.tensor.snap(rtp, donate=True, min_val=1, max_val=113)

            # Slot order: even-t slots (odd s) first - they need only mta;
            # odd-t slots run last, after the mtb shift-copies are built.
            order = [s for s in range(NSLOT) if s % 2 == 1] + [
                s for s in range(NSLOT) if s % 2 == 0
            ]

            # h0 max prefetch: slot order[0] is staircased through phase-1
            # chunk production (sub-range max passes as chunks land), the
            # next slots get full h0 passes spread over later kops so each
            # interleaved DVE doubling is data-ready in FIFO order.
            stair = {2: (0, 6), 4: (6, 10), 6: (10, 14), 8: (14, 18), 9: (18, 19)}
            pref_after = {9: (1,), 11: (2,), 13: (3,), 15: (4,), 17: (5,)}
            mx_pre = {}

            def emit_tt(si, h, c0=0, c1=NH, m3=None):
                """Emit the max pass for slot order[si], chunk-half h,
                chunk range [c0, c1). Returns the mx tile (3d view)."""
                s = order[si]
                par = (s + 1) % 2  # t parity; even t -> mta, odd t -> mtb
                if m3 is None:
                    pool = xpool0 if h == 0 else xpool1
                    mx = pool.tile([128, NH * 256], fp16, tag=f"mx{h}")
                    m3 = mx[:, :].rearrange("p (c w) -> p c w", w=256)
                src3 = mta3 if par == 0 else mtb3
                off = vt0 + (s - par)
                nc.vector.tensor_tensor(
                    m3[:, c0:c1, :],
                    mta3[h][:, c0:c1, 0:256],
                    src3[h][:, c0:c1, bass.ds(off, 256)],
                    Alu.max,
                )
                return m3

            # Phase 1: MT = M^T in (ko-chunk, i) layout, i doubled to 384.
            # Per kop (2 ko chunks): 8 matmuls -> mp psum, ACT copies the
            # 256-wide blocks out, DVE duplicates i 0..127 to 256..384.
            for kop in range(NCH // 2):
                ko0 = 2 * kop
                if kop == 0:
                    tsb = tsb0
                else:
                    tsb = tpool.tile([128, 1024], fp16, tag="tsb")
                    nc.sync.dma_start(
                        tsb[:, :], tt_d[:, ko0 * 512 : (ko0 + 2) * 512]
                    )
                mp = mpsum.tile([128, 512], fp32)
                for k2 in range(2):
                    for cc in range(4):
                        nc.tensor.matmul(
                            mp[:, k2 * 256 : (k2 + 1) * 256],
                            tsb[:, (k2 * 4 + cc) * 128 : (k2 * 4 + cc + 1) * 128],
                            xt[:, cc * 256 : (cc + 1) * 256],
                            start=(cc == 0),
                            stop=(cc == 3),
                        )
                mp3 = mp[:, :].rearrange("p (k w) -> p k w", k=2)
                h0, kh0 = divmod(ko0, NH)
                h1, kh1 = divmod(ko0 + 1, NH)
                ui = mybir.dt.uint32
                if h0 == h1:
                    # one strided copy for both chunks, one int32 doubling
                    nc.scalar.copy(
                        mta3[h0][:, kh0 : kh0 + 2, 0:256], mp3[:, :, :]
                    )
                    nc.vector.tensor_copy(
                        mta3[h0][:, kh0 : kh0 + 2, 256:384].bitcast(ui),
                        mta3[h0][:, kh0 : kh0 + 2, 0:128].bitcast(ui),
                    )
                else:  # kop 9 spans the two half tiles
                    for k2, (h, kh) in enumerate(((h0, kh0), (h1, kh1))):
                        nc.scalar.copy(
                            mta3[h][:, kh : kh + 1, 0:256], mp3[:, k2 : k2 + 1, :]
                        )
                        nc.vector.tensor_copy(
                            mta3[h][:, kh : kh + 1, 256:384].bitcast(ui),
                            mta3[h][:, kh : kh + 1, 0:128].bitcast(ui),
                        )
                if kop == 2:
                    # row-sum corrections from host-precomputed k-summed T:
                    # sa[o, i] = sum_k M[i, k, o] = (TS.T @ x.T)[o, i]
                    sa_ps = d2psum.tile([64, 256], fp32, tag="d2")
                    for cc in range(4):
                        nc.tensor.matmul(
                            sa_ps[:, :],
                            tsum[:, cc * 64 : (cc + 1) * 64],
                            xt[:, cc * 256 : (cc + 1) * 256],
                            start=(cc == 0),
                            stop=(cc == 3),
                        )
                    nc.scalar.copy(sa2[:, 0:256], sa_ps[:, :])
                    nc.scalar.copy(sa2[:, 256:512], sa_ps[:, :])
                if kop in stair:
                    c0, c1 = stair[kop]
                    mx_pre[0] = emit_tt(0, 0, c0, c1, mx_pre.get(0))
                if kop in pref_after:
                    for si in pref_after[kop]:
                        mx_pre[si] = emit_tt(si, 0)

            # odd-shift alignment copies, spread over the first 8 slots on
            # ACT: mtb[.., j] = mta[.., j+1]
            mtb_jobs = [(h, kh) for h in (0, 1) for kh in range(NH)]

            def emit_mtb(n):
                while n > 0 and mtb_jobs:
                    h, kh = mtb_jobs.pop(0)
                    ba = kh * CWA
                    nc.scalar.copy(
                        mtb[h][:, ba : ba + 383], mta[h][:, ba + 1 : ba + 384]
                    )
                    n -= 1

            # Phase 2, per shift slot s (t = t0 + s):
            #   2 DVE max instrs (fp16 2x, one per chunk-half)
            #   -> PE: 19 column-tiled matmul pairs (even half -> psum rows
            #   0:64, odd half -> 64:128), ACT copies /16 to fp16, PE fold
            #   matmul adds the halves + 2 row-sum corrections -> ACT
            #   exp(scale=-32, bias) -> PE accumulates e into self/pair.
            # Staggered PE pipeline: per slot, the queue gets
            #   lo-run(s) | fold+corr(s-1) | acc(s-2) | hi-run(s)
            # so no matmul ever heads the queue waiting on an ACT result
            # (dsb of s-1 and exp of s-2 are long done by the time their
            # consumers are reached), and the lo/hi runs on opposite array
            # column halves stay adjacent enough to overlap.
            def emit_fc(si):
                """fold + corrections for slot order[si] -> dp2, exp -> e."""
                s = order[si]
                dsb = state[si]["dsb"]
                dp2 = d2psum.tile([64, 256], fp32, tag="d2")
                nc.tensor.matmul(
                    dp2[:, :], ss[:, 0:64], dsb[:, :], start=True, stop=False
                )
                nc.tensor.matmul(
                    dp2[:, :], nh[:, :], sa2[:, 0:256], start=False, stop=False
                )
                nc.tensor.matmul(
                    dp2[:, :],
                    nh[:, :],
                    sa2[:, bass.ds(vp0 + s, 256)],
                    start=False,
                    stop=True,
                )
                e = epool.tile([64, 256], bf16, tag="e")
                nc.scalar.activation(
                    e[:, :],
                    dp2[:, :],
                    Act.Exp,
                    bias=bias[:, s : s + 1],
                    scale=-2.0 / DSB_SCALE,
                )
                state[si]["e"] = e

            def emit_ac(si):
                """accumulate e of slot order[si] into self/pair psums."""
                s = order[si]
                e = state[si]["e"]
                nc.tensor.matmul(
                    ps_self[:, :],
                    ident[:, :],
                    e[:, :],
                    start=(si == 0),
                    stop=(si == NSLOT - 1),
                )
                nc.tensor.matmul(
                    ps_pair[:, bass.ds(vp0 + s, 256)],
                    ident[:, :],
                    e[:, :],
                    start=False,
                    stop=(si == NSLOT - 1),
                    skip_group_check=True,
                )

            state = {}
            for si, s in enumerate(order):
                if si in mx_pre:
                    m3a = mx_pre.pop(si)
                else:
                    m3a = emit_tt(si, 0)
                if si == NSLOT - 1:
                    # split the last h1 max so its hi matmuls start earlier
                    m3b = emit_tt(si, 1, 0, 10)
                    emit_tt(si, 1, 10, NH, m3b)
                else:
                    m3b = emit_tt(si, 1)
                if si >= 1:
                    emit_fc(si - 1)
                if si >= 2:
                    emit_ac(si - 2)
                dp = dpsum.tile([128, 256], fp32, tag="dp")
                # interleaved column-tiled pairs: lo/hi matmuls adjacent in
                # the queue run concurrently in opposite array halves
                for c in range(NH):
                    nc.tensor.matmul(
                        dp[0:64, :],
                        ss[:, 0:64],
                        m3a[:, c, :],
                        start=(c == 0),
                        stop=(c == NH - 1),
                        tile_position=(0, 0),
                    )
                    nc.tensor.matmul(
                        dp[64:128, :],
                        ss[:, 0:64],
                        m3b[:, c, :],
                        start=(c == 0 and START_HI),
                        stop=(c == NH - 1),
                        tile_position=(0, 64),
                        skip_group_check=True,
                    )
                dsb = spool.tile([128, 256], fp16, tag="dsb")
                nc.scalar.mul(dsb[:, :], dp[:, :], DSB_SCALE)
                state[si] = {"dsb": dsb}
                if si < 8:
                    emit_mtb(5)
            emit_fc(NSLOT - 1)
            emit_ac(NSLOT - 2)
            emit_ac(NSLOT - 1)

            # raw accumulators out; the host combine (which already sums
            # the 8 cores) folds pair[0:256]+pair[256:512]+self
            outsb = cpool.tile([64, 768], fp32)
            nc.scalar.copy(outsb[:, 0:512], ps_pair[:, :])
            nc.scalar.copy(outsb[:, 512:768], ps_self[:, :])
            nc.sync.dma_start(out_d[:, :], outsb[:, :])

    nc.compile()
    return nc


def get_nc():
    if "nc" not in _NC_CACHE:
        _NC_CACHE["nc"] = _build_nc()
    return _NC_CACHE["nc"]


def host_inputs(x, T):
    """Host-side shard prep: returns the 8 per-core input maps."""
    x = np.asarray(x, dtype=np.float32)
    T = np.asarray(T, dtype=np.float32)
    T2p = np.zeros((F, KOP), np.float32)
    T2p[:, :KO] = T.reshape(F, KO)
    # tt[p, ko*512 + cc*128 + j] = T2p[cc*128+p, ko*128+j]
    tt = (
        np.ascontiguousarray(
            T2p.reshape(4, 128, NCH, 128).transpose(1, 2, 0, 3)
        )
        .reshape(128, NCH * 512)
        .astype(np.float16)
    )
    # xt[p, cc*256 + i] = x[i, cc*128+p]
    xt = (
        np.ascontiguousarray(x.T.reshape(4, 128, B).transpose(1, 0, 2))
        .reshape(128, 1024)
        .astype(np.float16)
    )
    # tsum[p, cc*64 + o] = TS[cc*128+p, o],  TS[f, o] = sum_k T[f, k, o]
    TS = T.reshape(F, K, O).sum(axis=1)
    tsum = (
        np.ascontiguousarray(TS.reshape(4, 128, O).transpose(1, 0, 2))
        .reshape(128, 256)
        .astype(np.float16)
    )
    ss = (np.arange(128)[:, None] % 64 == np.arange(64)[None, :]).astype(
        np.float16
    )
    import ml_dtypes
    nh = (-0.5 * DSB_SCALE * np.eye(64)).astype(np.float16)
    ident = np.eye(64).astype(ml_dtypes.bfloat16)
    in_maps = []
    for c in range(8):
        offs = np.array([[16 * c + 1, 0]], np.int32)
        biases = np.zeros((64, NSLOT), np.float32)
        if c == 7:
            biases[:, 15] = -LN2  # t = 128: every pair covered twice
        in_maps.append(
            {
                "xt": xt,
                "tt": tt,
                "tsum": tsum,
                "ssel": ss,
                "nhalf": nh,
                "ident": ident,
                "offs": offs,
                "bias": biases,
            }
        )
    return in_maps


def combine(results):
    """Sum per-core partial outputs [64,256] -> full [256,64] fp32.

    The reference computes sum_j exp(-d) (including the j=i term, = 1.0) in
    fp32 and then subtracts 1.0. Replicate those fp32 semantics exactly: the
    off-diagonal terms here are ~1e-25 and are fully absorbed by the +1.
    """
    acc = np.zeros((64, 256), np.float64)
    for r in results:
        o = r["out"].astype(np.float64)
        acc += o[:, 0:256] + o[:, 256:512] + o[:, 512:768]
    full = np.ascontiguousarray(acc.T).astype(np.float32)
    return (np.float32(1.0) + full) - np.float32(1.0)


def run_on_hw(x, T, trace=False):
    from concourse.bass_utils import run_bass_kernel_spmd

    nc = get_nc()
    in_maps = host_inputs(x, T)
    res = run_bass_kernel_spmd(
        nc, in_maps, core_ids=list(range(8)), trace=trace
    )
    return combine(res.results), res


def kernel(x, T):
    out, _ = run_on_hw(x, T, trace=False)
    return out


# revision 27
# speedup vs baseline: 1.0419x; 1.0276x over previous
"""MinibatchDiscrimination Bass kernel for 8 TRN2 NeuronCores.

out[i,o] = sum_{j!=i} exp(-sum_k |M[i,k,o]-M[j,k,o]|),  M = x @ T.

Strategy: cyclic-offset pairing over the symmetric BxB distance matrix.
Shift t pairs row i with row (i+t) mod B; t=1..B/2 covers every unordered
pair exactly once (t=B/2 twice -> halved via an exp bias of -ln2). Core c
computes shifts t in [16c+1, 16c+16]; each pair contributes to both rows.
Host sums the 8 partial outputs with the reference's fp32 `(1+s)-1`
absorption.

Key identity (one DVE pass per shift):
    sum_k |a_k - b_k| = 2*sum_k max(a_k, b_k) - sum_k a_k - sum_k b_k
The k-reduction of the max tensor runs on the PE as column-tiled matmul
pairs (two 64-wide selection matmuls concurrent in separate array column
halves: even half over chunks 0..18, odd half over 19..37), then one fold
matmul adds the two PSUM halves. Row-sum corrections use a host-precomputed
k-summed T (sa = x @ TS), removing the per-chunk row-sum matmuls entirely.

Schedule notes (from perfetto analysis; 146us baseline -> ~124-131us):
 - the DVE max pass is the hard floor (~5.2us/shift, fp16 2x tensor_tensor
   is the DVE cap); all other engines are kept off its SBUF port (GPSIMD
   fully idle: it shares the DVE port and was degrading the maxes to 1x),
 - max instrs are merged to one per chunk-half (2/shift instead of 4),
 - the first shift's h0 max is staircased through phase-1 chunk
   production and the next five shifts' h0 maxes are prefetched, so the
   DVE streams during the tail of the M matmul and never stalls after,
 - per-kop chunk copies are single strided ACT instrs; the i-doubling
   copies ride the DVE queue as int32 copies ahead of the first max; the
   odd-shift alignment copies (mtb) run on ACT spread over early shifts,
 - fold+corrections (shift-1) and e-accumulation (shift-2) matmuls are
   staggered between the interleaved column-pair runs so no PE
   instruction heads the queue waiting on an ACT result,
 - PSUM matmul start=True clears are partition-range-scoped (measured),
   so both column-half groups carry their own start.
"""

import numpy as np

B = 256
F = 512
K = 75
O = 64
KO = K * O          # 4800
KOP = 4864          # padded to 38*128
NCH = KOP // 128    # 38 ko-chunks
NH = NCH // 2       # 19 chunks per half
CWA = 384           # chunk width: M^T[.., i] doubled to i in [0,384)
NSLOT = 16
LN2 = float(np.log(2.0))
DSB_SCALE = 1.0 / 16.0   # fp16 pre-scale of the max-sum before the fold
START_HI = True          # PSUM start=True clears are partition-scoped
                         # (measured): each column-half group needs its own

_NC_CACHE = {}


def _build_nc():
    import concourse.bacc as bacc
    import concourse.bass as bass
    import concourse.mybir as mybir
    from concourse import tile

    fp16 = mybir.dt.float16
    fp32 = mybir.dt.float32
    bf16 = mybir.dt.bfloat16
    i32 = mybir.dt.int32
    Alu = mybir.AluOpType
    Act = mybir.ActivationFunctionType

    nc = bacc.Bacc(
        "TRN2", target_bir_lowering=False, debug=False, num_devices=8
    )

    with tile.TileContext(nc) as tc:
        xt_d = nc.dram_tensor("xt", [128, 1024], fp16, kind="ExternalInput")
        tt_d = nc.dram_tensor("tt", [128, NCH * 512], fp16, kind="ExternalInput")
        ts_d = nc.dram_tensor("tsum", [128, 256], fp16, kind="ExternalInput")
        ss_d = nc.dram_tensor("ssel", [128, 64], fp16, kind="ExternalInput")
        nh_d = nc.dram_tensor("nhalf", [64, 64], fp16, kind="ExternalInput")
        id_d = nc.dram_tensor("ident", [64, 64], bf16, kind="ExternalInput")
        of_d = nc.dram_tensor("offs", [1, 2], i32, kind="ExternalInput")
        bi_d = nc.dram_tensor("bias", [64, NSLOT], fp32, kind="ExternalInput")
        out_d = nc.dram_tensor("out", [64, 768], fp32, kind="ExternalOutput")

        with (
            tc.tile_pool(name="const", bufs=1) as cpool,
            tc.tile_pool(name="tload", bufs=3) as tpool,
            tc.tile_pool(name="mxh0", bufs=7) as xpool0,
            tc.tile_pool(name="mxh1", bufs=3) as xpool1,
            tc.tile_pool(name="dsb", bufs=2) as spool,
            tc.tile_pool(name="esb", bufs=3) as epool,
            tc.tile_pool(name="mpsum", bufs=2, space="PSUM") as mpsum,
            tc.tile_pool(name="dpsum", bufs=2, space="PSUM") as dpsum,
            tc.tile_pool(name="d2psum", bufs=2, space="PSUM") as d2psum,
            tc.tile_pool(name="apsum", bufs=1, space="PSUM") as apsum,
        ):
            # prefetch the first T chunk-pair first so the first matmul can
            # start as early as possible; xt as one row-contiguous DMA
            tsb0 = tpool.tile([128, 1024], fp16, tag="tsb")
            nc.sync.dma_start(tsb0[:, :], tt_d[:, 0:1024])
            xt = cpool.tile([128, 1024], fp16)
            nc.sync.dma_start(xt[:, :], xt_d[:, :])
            tsum = cpool.tile([128, 256], fp16)
            nc.sync.dma_start(tsum[:, :], ts_d[:, :])
            offs = cpool.tile([1, 2], i32)
            nc.sync.dma_start(offs[:, :], of_d[:, :])
            ss = cpool.tile([128, 64], fp16)
            nc.sync.dma_start(ss[:, :], ss_d[:, :])
            nh = cpool.tile([64, 64], fp16)
            nc.sync.dma_start(nh[:, :], nh_d[:, :])
            ident = cpool.tile([64, 64], bf16)
            nc.sync.dma_start(ident[:, :], id_d[:, :])
            bias = cpool.tile([64, NSLOT], fp32)
            nc.sync.dma_start(bias[:, :], bi_d[:, :])

            # chunk-half tiles: [0]=chunks 0..18, [1]=chunks 19..37
            mta = [
                cpool.tile([128, NH * CWA], fp16, name=f"mta{h}", tag=f"mta{h}")
                for h in (0, 1)
            ]
            mtb = [
                cpool.tile([128, NH * CWA], fp16, name=f"mtb{h}", tag=f"mtb{h}")
                for h in (0, 1)
            ]
            mta3 = [t[:, :].rearrange("p (c w) -> p c w", w=CWA) for t in mta]
            mtb3 = [t[:, :].rearrange("p (c w) -> p c w", w=CWA) for t in mtb]
            sa2 = cpool.tile([64, 512], fp16)

            ps_self = apsum.tile([64, 256], fp32, tag="pself")
            ps_pair = apsum.tile([64, 512], fp32, tag="ppair")
            nc.vector.memset(ps_pair[:, :], 0.0)

            # one register load of t0 = 16*core + 1 per engine
            rtv = nc.vector.alloc_register("t0v")
            nc.vector.reg_load(rtv, offs[0:1, 0:1])
            vt0 = nc.vector.snap(rtv, donate=True, min_val=1, max_val=113)
            rtp = nc.tensor.alloc_register("t0p")
            nc.tensor.reg_load(rtp, offs[0:1, 0:1])
            vp0 = nc.tensor.snap(rtp, donate=True, min_val=1, max_val=113)

            # Slot order: even-t slots (odd s) first - they need only mta;
            # odd-t slots run last, after the mtb shift-copies are built.
            order = [s for s in range(NSLOT) if s % 2 == 1] + [
                s for s in range(NSLOT) if s % 2 == 0
            ]

            # h0 max prefetch: slot order[0] is staircased through phase-1
            # chunk production (sub-range max passes as chunks land), the
            # next slots get full h0 passes spread over later kops so each
            # interleaved DVE doubling is data-ready in FIFO order.
            stair = {2: (0, 6), 4: (6, 10), 6: (10, 14), 8: (14, 18), 9: (18, 19)}
            pref_after = {9: (1,), 11: (2,), 13: (3,), 15: (4,), 17: (5,)}
            mx_pre = {}

            def emit_tt(si, h, c0=0, c1=NH, m3=None):
                """Emit the max pass for slot order[si], chunk-half h,
                chunk range [c0, c1). Returns the mx tile (3d view)."""
                s = order[si]
                par = (s + 1) % 2  # t parity; even t -> mta, odd t -> mtb
                if m3 is None:
                    pool = xpool0 if h == 0 else xpool1
                    mx = pool.tile([128, NH * 256], fp16, tag=f"mx{h}")
                    m3 = mx[:, :].rearrange("p (c w) -> p c w", w=256)
                src3 = mta3 if par == 0 else mtb3
                off = vt0 + (s - par)
                nc.vector.tensor_tensor(
                    m3[:, c0:c1, :],
                    mta3[h][:, c0:c1, 0:256],
                    src3[h][:, c0:c1, bass.ds(off, 256)],
                    Alu.max,
                )
                return m3

            # Phase 1: MT = M^T in (ko-chunk, i) layout, i doubled to 384.
            # Per kop (2 ko chunks): 8 matmuls -> mp psum, ACT copies the
            # 256-wide blocks out, DVE duplicates i 0..127 to 256..384.
            for kop in range(NCH // 2):
                ko0 = 2 * kop
                if kop == 0:
                    tsb = tsb0
                else:
                    tsb = tpool.tile([128, 1024], fp16, tag="tsb")
                    nc.sync.dma_start(
                        tsb[:, :], tt_d[:, ko0 * 512 : (ko0 + 2) * 512]
                    )
                mp = mpsum.tile([128, 512], fp32)
                for k2 in range(2):
                    for cc in range(4):
                        nc.tensor.matmul(
                            mp[:, k2 * 256 : (k2 + 1) * 256],
                            tsb[:, (k2 * 4 + cc) * 128 : (k2 * 4 + cc + 1) * 128],
                            xt[:, cc * 256 : (cc + 1) * 256],
                            start=(cc == 0),
                            stop=(cc == 3),
                        )
                mp3 = mp[:, :].rearrange("p (k w) -> p k w", k=2)
                h0, kh0 = divmod(ko0, NH)
                h1, kh1 = divmod(ko0 + 1, NH)
                ui = mybir.dt.uint32
                if h0 == h1:
                    # one strided copy for both chunks, one int32 doubling
                    nc.scalar.copy(
                        mta3[h0][:, kh0 : kh0 + 2, 0:256], mp3[:, :, :]
                    )
                    nc.vector.tensor_copy(
                        mta3[h0][:, kh0 : kh0 + 2, 256:384].bitcast(ui),
                        mta3[h0][:, kh0 : kh0 + 2, 0:128].bitcast(ui),
                    )
                else:  # kop 9 spans the two half tiles
                    for k2, (h, kh) in enumerate(((h0, kh0), (h1, kh1))):
                        nc.scalar.copy(
                            mta3[h][:, kh : kh + 1, 0:256], mp3[:, k2 : k2 + 1, :]
                        )
                        nc.vector.tensor_copy(
                            mta3[h][:, kh : kh + 1, 256:384].bitcast(ui),
                            mta3[h][:, kh : kh + 1, 0:128].bitcast(ui),
                        )
                if kop == 2:
                    # row-sum corrections from host-precomputed k-summed T:
                    # sa[o, i] = sum_k M[i, k, o] = (TS.T @ x.T)[o, i]
                    sa_ps = d2psum.tile([64, 256], fp32, tag="d2")
                    for cc in range(4):
                        nc.tensor.matmul(
                            sa_ps[:, :],
                            tsum[:, cc * 64 : (cc + 1) * 64],
                            xt[:, cc * 256 : (cc + 1) * 256],
                            start=(cc == 0),
                            stop=(cc == 3),
                        )
                    nc.scalar.copy(sa2[:, 0:256], sa_ps[:, :])
                    nc.scalar.copy(sa2[:, 256:512], sa_ps[:, :])
                if kop in stair:
                    c0, c1 = stair[kop]
                    mx_pre[0] = emit_tt(0, 0, c0, c1, mx_pre.get(0))
                if kop in pref_after:
                    for si in pref_after[kop]:
                        mx_pre[si] = emit_tt(si, 0)

            # odd-shift alignment copies, spread over the first 8 slots on
            # ACT: mtb[.., j] = mta[.., j+1]
            mtb_jobs = [(h, kh) for h in (0, 1) for kh in range(NH)]

            def emit_mtb(n):
                while n > 0 and mtb_jobs:
                    h, kh = mtb_jobs.pop(0)
                    ba = kh * CWA
                    nc.scalar.copy(
                        mtb[h][:, ba : ba + 383], mta[h][:, ba + 1 : ba + 384]
                    )
                    n -= 1

            # Phase 2, per shift slot s (t = t0 + s):
            #   2 DVE max instrs (fp16 2x, one per chunk-half)
            #   -> PE: 19 column-tiled matmul pairs (even half -> psum rows
            #   0:64, odd half -> 64:128), ACT copies /16 to fp16, PE fold
            #   matmul adds the halves + 2 row-sum corrections -> ACT
            #   exp(scale=-32, bias) -> PE accumulates e into self/pair.
            # Staggered PE pipeline: per slot, the queue gets
            #   lo-run(s) | fold+corr(s-1) | acc(s-2) | hi-run(s)
            # so no matmul ever heads the queue waiting on an ACT result
            # (dsb of s-1 and exp of s-2 are long done by the time their
            # consumers are reached), and the lo/hi runs on opposite array
            # column halves stay adjacent enough to overlap.
            def emit_fc(si):
                """fold + corrections for slot order[si] -> dp2, exp -> e."""
                s = order[si]
                dsb = state[si]["dsb"]
                dp2 = d2psum.tile([64, 256], fp32, tag="d2")
                nc.tensor.matmul(
                    dp2[:, :], ss[:, 0:64], dsb[:, :], start=True, stop=False
                )
                nc.tensor.matmul(
                    dp2[:, :], nh[:, :], sa2[:, 0:256], start=False, stop=False
                )
                nc.tensor.matmul(
                    dp2[:, :],
                    nh[:, :],
                    sa2[:, bass.ds(vp0 + s, 256)],
                    start=False,
                    stop=True,
                )
                e = epool.tile([64, 256], bf16, tag="e")
                nc.scalar.activation(
                    e[:, :],
                    dp2[:, :],
                    Act.Exp,
                    bias=bias[:, s : s + 1],
                    scale=-2.0 / DSB_SCALE,
                )
                state[si]["e"] = e

            def emit_ac(si):
                """accumulate e of slot order[si] into self/pair psums."""
                s = order[si]
                e = state[si]["e"]
                nc.tensor.matmul(
                    ps_self[:, :],
                    ident[:, :],
                    e[:, :],
                    start=(si == 0),
                    stop=(si == NSLOT - 1),
                )
                nc.tensor.matmul(
                    ps_pair[:, bass.ds(vp0 + s, 256)],
                    ident[:, :],
                    e[:, :],
                    start=False,
                    stop=(si == NSLOT - 1),
                    skip_group_check=True,
                )

            state = {}
            for si, s in enumerate(order):
                if si in mx_pre:
                    m3a = mx_pre.pop(si)
                else:
                    m3a = emit_tt(si, 0)
                if si == NSLOT - 1:
                    # split the last h1 max so its hi matmuls start earlier
                    m3b = emit_tt(si, 1, 0, 10)
                    emit_tt(si, 1, 10, NH, m3b)
                else:
                    m3b = emit_tt(si, 1)
                if si >= 1:
                    emit_fc(si - 1)
                if si >= 2:
                    emit_ac(si - 2)
                dp = dpsum.tile([128, 256], fp32, tag="dp")
                # interleaved column-tiled pairs: lo/hi matmuls adjacent in
                # the queue run concurrently in opposite array halves
                for c in range(NH):
                    nc.tensor.matmul(
                        dp[0:64, :],
                        ss[:, 0:64],
                        m3a[:, c, :],
                        start=(c == 0),
                        stop=(c == NH - 1),
                        tile_position=(0, 0),
                    )
                    nc.tensor.matmul(
                        dp[64:128, :],
                        ss[:, 0:64],
                        m3b[:, c, :],
                        start=(c == 0 and START_HI),
                        stop=(c == NH - 1),
                        tile_position=(0, 64),
                        skip_group_check=True,
                    )
                dsb = spool.tile([128, 256], fp16, tag="dsb")
                nc.scalar.mul(dsb[:, :], dp[:, :], DSB_SCALE)
                state[si] = {"dsb": dsb}
                if si < 8:
                    emit_mtb(5)
            emit_fc(NSLOT - 1)
            emit_ac(NSLOT - 2)
            emit_ac(NSLOT - 1)

            # raw accumulators out; the host combine (which already sums
            # the 8 cores) folds pair[0:256]+pair[256:512]+self
            outsb = cpool.tile([64, 768], fp32)
            nc.scalar.copy(outsb[:, 0:512], ps_pair[:, :])
            nc.scalar.copy(outsb[:, 512:768], ps_self[:, :])
            nc.sync.dma_start(out_d[:, :], outsb[:, :])

    nc.compile()
    return nc


def get_nc():
    if "nc" not in _NC_CACHE:
        _NC_CACHE["nc"] = _build_nc()
    return _NC_CACHE["nc"]


def host_inputs(x, T):
    """Host-side shard prep: returns the 8 per-core input maps."""
    x = np.asarray(x, dtype=np.float32)
    T = np.asarray(T, dtype=np.float32)
    T2p = np.zeros((F, KOP), np.float32)
    T2p[:, :KO] = T.reshape(F, KO)
    # tt[p, ko*512 + cc*128 + j] = T2p[cc*128+p, ko*128+j]
    tt = (
        np.ascontiguousarray(
            T2p.reshape(4, 128, NCH, 128).transpose(1, 2, 0, 3)
        )
        .reshape(128, NCH * 512)
        .astype(np.float16)
    )
    # xt[p, cc*256 + i] = x[i, cc*128+p]
    xt = (
        np.ascontiguousarray(x.T.reshape(4, 128, B).transpose(1, 0, 2))
        .reshape(128, 1024)
        .astype(np.float16)
    )
    # tsum[p, cc*64 + o] = TS[cc*128+p, o],  TS[f, o] = sum_k T[f, k, o]
    TS = T.reshape(F, K, O).sum(axis=1)
    tsum = (
        np.ascontiguousarray(TS.reshape(4, 128, O).transpose(1, 0, 2))
        .reshape(128, 256)
        .astype(np.float16)
    )
    ss = (np.arange(128)[:, None] % 64 == np.arange(64)[None, :]).astype(
        np.float16
    )
    import ml_dtypes
    nh = (-0.5 * DSB_SCALE * np.eye(64)).astype(np.float16)
    ident = np.eye(64).astype(ml_dtypes.bfloat16)
    in_maps = []
    for c in range(8):
        offs = np.array([[16 * c + 1, 0]], np.int32)
        biases = np.zeros((64, NSLOT), np.float32)
        if c == 7:
            biases[:, 15] = -LN2  # t = 128: every pair covered twice
        in_maps.append(
            {
                "xt": xt,
                "tt": tt,
                "tsum": tsum,
                "ssel": ss,
                "nhalf": nh,
                "ident": ident,
                "offs": offs,
                "bias": biases,
            }
        )
    return in_maps


def combine(results):
    """Sum per-core partial outputs [64,256] -> full [256,64] fp32.

    The reference computes sum_j exp(-d) (including the j=i term, = 1.0) in
    fp32 and then subtracts 1.0. Replicate those fp32 semantics exactly: the
    off-diagonal terms here are ~1e-25 and are fully absorbed by the +1.
    """
    acc = np.zeros((64, 256), np.float64)
    for r in results:
        o = r["out"].astype(np.float64)
        acc += o[:, 0:256] + o[:, 256:512] + o[:, 512:768]
    full = np.ascontiguousarray(acc.T).astype(np.float32)
    return (np.float32(1.0) + full) - np.float32(1.0)


def run_on_hw(x, T, trace=False):
    from concourse.bass_utils import run_bass_kernel_spmd

    nc = get_nc()
    in_maps = host_inputs(x, T)
    res = run_bass_kernel_spmd(
        nc, in_maps, core_ids=list(range(8)), trace=trace
    )
    return combine(res.results), res


def kernel(x, T):
    out, _ = run_on_hw(x, T, trace=False)
    return out


# revision 28
# speedup vs baseline: 1.0548x; 1.0124x over previous
"""MinibatchDiscrimination Bass kernel for 8 TRN2 NeuronCores.

out[i,o] = sum_{j!=i} exp(-sum_k |M[i,k,o]-M[j,k,o]|),  M = x @ T.

Strategy: cyclic-offset pairing over the symmetric BxB distance matrix.
Shift t pairs row i with row (i+t) mod B; t=1..B/2 covers every unordered
pair exactly once (t=B/2 twice -> halved via an exp bias of -ln2). Core c
computes shifts t in [16c+1, 16c+16]; each pair contributes to both rows.
Host sums the 8 partial outputs with the reference's fp32 `(1+s)-1`
absorption.

Key identity (one DVE pass per shift):
    sum_k |a_k - b_k| = 2*sum_k max(a_k, b_k) - sum_k a_k - sum_k b_k
The k-reduction of the max tensor runs on the PE as column-tiled matmul
pairs (two 64-wide selection matmuls concurrent in separate array column
halves: even half over chunks 0..18, odd half over 19..37), then one fold
matmul adds the two PSUM halves. Row-sum corrections use a host-precomputed
k-summed T (sa = x @ TS), removing the per-chunk row-sum matmuls entirely.

Schedule notes (from perfetto analysis; 146us baseline -> ~124-131us):
 - the DVE max pass is the hard floor (~5.2us/shift, fp16 2x tensor_tensor
   is the DVE cap); all other engines are kept off its SBUF port (GPSIMD
   fully idle: it shares the DVE port and was degrading the maxes to 1x),
 - max instrs are merged to one per chunk-half (2/shift instead of 4),
 - the first shift's h0 max is staircased through phase-1 chunk
   production and the next five shifts' h0 maxes are prefetched, so the
   DVE streams during the tail of the M matmul and never stalls after,
 - per-kop chunk copies are single strided ACT instrs; the i-doubling
   copies ride the DVE queue as int32 copies ahead of the first max; the
   odd-shift alignment copies (mtb) run on ACT spread over early shifts,
 - fold+corrections (shift-1) and e-accumulation (shift-2) matmuls are
   staggered between the interleaved column-pair runs so no PE
   instruction heads the queue waiting on an ACT result,
 - PSUM matmul start=True clears are partition-range-scoped (measured),
   so both column-half groups carry their own start.
"""

import numpy as np

B = 256
F = 512
K = 75
O = 64
KO = K * O          # 4800
KOP = 4864          # padded to 38*128
NCH = KOP // 128    # 38 ko-chunks
NH = NCH // 2       # 19 chunks per half
CWA = 384           # chunk width: M^T[.., i] doubled to i in [0,384)
NSLOT = 16
LN2 = float(np.log(2.0))
DSB_SCALE = 1.0 / 16.0   # fp16 pre-scale of the max-sum before the fold
START_HI = True          # PSUM start=True clears are partition-scoped
                         # (measured): each column-half group needs its own

_NC_CACHE = {}


def _build_nc():
    import concourse.bacc as bacc
    import concourse.bass as bass
    import concourse.mybir as mybir
    from concourse import tile

    fp16 = mybir.dt.float16
    fp32 = mybir.dt.float32
    bf16 = mybir.dt.bfloat16
    i32 = mybir.dt.int32
    Alu = mybir.AluOpType
    Act = mybir.ActivationFunctionType

    nc = bacc.Bacc(
        "TRN2", target_bir_lowering=False, debug=False, num_devices=8
    )

    with tile.TileContext(nc) as tc:
        xt_d = nc.dram_tensor("xt", [128, 1024], fp16, kind="ExternalInput")
        tt_d = nc.dram_tensor("tt", [128, NCH * 512], fp16, kind="ExternalInput")
        ts_d = nc.dram_tensor("tsum", [128, 256], fp16, kind="ExternalInput")
        ss_d = nc.dram_tensor("ssel", [128, 64], fp16, kind="ExternalInput")
        nh_d = nc.dram_tensor("nhalf", [64, 64], fp16, kind="ExternalInput")
        id_d = nc.dram_tensor("ident", [64, 64], bf16, kind="ExternalInput")
        of_d = nc.dram_tensor("offs", [1, 2], i32, kind="ExternalInput")
        bi_d = nc.dram_tensor("bias", [64, NSLOT], fp32, kind="ExternalInput")
        out_d = nc.dram_tensor("out", [64, 768], fp32, kind="ExternalOutput")

        with (
            tc.tile_pool(name="const", bufs=1) as cpool,
            tc.tile_pool(name="tload", bufs=3) as tpool,
            tc.tile_pool(name="mxh0", bufs=7) as xpool0,
            tc.tile_pool(name="mxh1", bufs=3) as xpool1,
            tc.tile_pool(name="dsb", bufs=2) as spool,
            tc.tile_pool(name="esb", bufs=3) as epool,
            tc.tile_pool(name="mpsum", bufs=2, space="PSUM") as mpsum,
            tc.tile_pool(name="dpsum", bufs=2, space="PSUM") as dpsum,
            tc.tile_pool(name="d2psum", bufs=2, space="PSUM") as d2psum,
            tc.tile_pool(name="apsum", bufs=1, space="PSUM") as apsum,
        ):
            # ss first (128B, it feeds the PE warm-up), then the first T
            # chunk-pair and xt so the first real matmul starts early
            ss = cpool.tile([128, 64], fp16)
            nc.sync.dma_start(ss[:, :], ss_d[:, :])
            tsb0 = tpool.tile([128, 1024], fp16, tag="tsb")
            nc.sync.dma_start(tsb0[:, :], tt_d[:, 0:1024])
            xt = cpool.tile([128, 1024], fp16)
            nc.sync.dma_start(xt[:, :], xt_d[:, :])
            tsum = cpool.tile([128, 256], fp16)
            nc.sync.dma_start(tsum[:, :], ts_d[:, :])
            offs = cpool.tile([1, 2], i32)
            nc.sync.dma_start(offs[:, :], of_d[:, :])

            # HAM warm-up: eight N=512 matmuls on a zeroed scratch tile
            # (~3.5us) while tsb0/xt stream in, so the PE clock gate is
            # already lifted to 2.4GHz when the real phase-1 matmuls
            # begin. Sized to finish as those DMAs land.
            wsrc = cpool.tile([128, 512], fp16, name="wsrc")
            nc.gpsimd.memset(wsrc[:, :], 0.0)
            warm = mpsum.tile([128, 512], fp32, tag="mp")
            for i in range(8):
                nc.tensor.matmul(
                    warm[0:64, :],
                    ss[:, 0:64],
                    wsrc[:, :],
                    start=(i == 0),
                    stop=(i == 7),
                )
            nh = cpool.tile([64, 64], fp16)
            nc.sync.dma_start(nh[:, :], nh_d[:, :])
            ident = cpool.tile([64, 64], bf16)
            nc.sync.dma_start(ident[:, :], id_d[:, :])
            bias = cpool.tile([64, NSLOT], fp32)
            nc.sync.dma_start(bias[:, :], bi_d[:, :])

            # chunk-half tiles: [0]=chunks 0..18, [1]=chunks 19..37
            mta = [
                cpool.tile([128, NH * CWA], fp16, name=f"mta{h}", tag=f"mta{h}")
                for h in (0, 1)
            ]
            mtb = [
                cpool.tile([128, NH * CWA], fp16, name=f"mtb{h}", tag=f"mtb{h}")
                for h in (0, 1)
            ]
            mta3 = [t[:, :].rearrange("p (c w) -> p c w", w=CWA) for t in mta]
            mtb3 = [t[:, :].rearrange("p (c w) -> p c w", w=CWA) for t in mtb]
            sa2 = cpool.tile([64, 512], fp16)

            ps_self = apsum.tile([64, 256], fp32, tag="pself")
            ps_pair = apsum.tile([64, 512], fp32, tag="ppair")
            nc.vector.memset(ps_pair[:, :], 0.0)

            # one register load of t0 = 16*core + 1 per engine
            rtv = nc.vector.alloc_register("t0v")
            nc.vector.reg_load(rtv, offs[0:1, 0:1])
            vt0 = nc.vector.snap(rtv, donate=True, min_val=1, max_val=113)
            rtp = nc.tensor.alloc_register("t0p")
            nc.tensor.reg_load(rtp, offs[0:1, 0:1])
            vp0 = nc.tensor.snap(rtp, donate=True, min_val=1, max_val=113)

            # Slot order: even-t slots (odd s) first - they need only mta;
            # odd-t slots run last, after the mtb shift-copies are built.
            order = [s for s in range(NSLOT) if s % 2 == 1] + [
                s for s in range(NSLOT) if s % 2 == 0
            ]

            # h0 max prefetch: slot order[0] is staircased through phase-1
            # chunk production (sub-range max passes as chunks land), the
            # next slots get full h0 passes spread over later kops so each
            # interleaved DVE doubling is data-ready in FIFO order.
            stair = {2: (0, 6), 4: (6, 10), 6: (10, 14), 8: (14, 18), 9: (18, 19)}
            pref_after = {9: (1,), 11: (2,), 13: (3,), 15: (4,), 17: (5,)}
            mx_pre = {}

            def emit_tt(si, h, c0=0, c1=NH, m3=None):
                """Emit the max pass for slot order[si], chunk-half h,
                chunk range [c0, c1). Returns the mx tile (3d view)."""
                s = order[si]
                par = (s + 1) % 2  # t parity; even t -> mta, odd t -> mtb
                if m3 is None:
                    pool = xpool0 if h == 0 else xpool1
                    mx = pool.tile([128, NH * 256], fp16, tag=f"mx{h}")
                    m3 = mx[:, :].rearrange("p (c w) -> p c w", w=256)
                src3 = mta3 if par == 0 else mtb3
                off = vt0 + (s - par)
                nc.vector.tensor_tensor(
                    m3[:, c0:c1, :],
                    mta3[h][:, c0:c1, 0:256],
                    src3[h][:, c0:c1, bass.ds(off, 256)],
                    Alu.max,
                )
                return m3

            # Phase 1: MT = M^T in (ko-chunk, i) layout, i doubled to 384.
            # Per kop (2 ko chunks): 8 matmuls -> mp psum, ACT copies the
            # 256-wide blocks out, DVE duplicates i 0..127 to 256..384.
            for kop in range(NCH // 2):
                ko0 = 2 * kop
                if kop == 0:
                    tsb = tsb0
                else:
                    tsb = tpool.tile([128, 1024], fp16, tag="tsb")
                    nc.sync.dma_start(
                        tsb[:, :], tt_d[:, ko0 * 512 : (ko0 + 2) * 512]
                    )
                mp = mpsum.tile([128, 512], fp32)
                for k2 in range(2):
                    for cc in range(4):
                        nc.tensor.matmul(
                            mp[:, k2 * 256 : (k2 + 1) * 256],
                            tsb[:, (k2 * 4 + cc) * 128 : (k2 * 4 + cc + 1) * 128],
                            xt[:, cc * 256 : (cc + 1) * 256],
                            start=(cc == 0),
                            stop=(cc == 3),
                        )
                mp3 = mp[:, :].rearrange("p (k w) -> p k w", k=2)
                h0, kh0 = divmod(ko0, NH)
                h1, kh1 = divmod(ko0 + 1, NH)
                ui = mybir.dt.uint32
                if h0 == h1:
                    # one strided copy for both chunks, one int32 doubling
                    nc.scalar.copy(
                        mta3[h0][:, kh0 : kh0 + 2, 0:256], mp3[:, :, :]
                    )
                    nc.vector.tensor_copy(
                        mta3[h0][:, kh0 : kh0 + 2, 256:384].bitcast(ui),
                        mta3[h0][:, kh0 : kh0 + 2, 0:128].bitcast(ui),
                    )
                else:  # kop 9 spans the two half tiles
                    for k2, (h, kh) in enumerate(((h0, kh0), (h1, kh1))):
                        nc.scalar.copy(
                            mta3[h][:, kh : kh + 1, 0:256], mp3[:, k2 : k2 + 1, :]
                        )
                        nc.vector.tensor_copy(
                            mta3[h][:, kh : kh + 1, 256:384].bitcast(ui),
                            mta3[h][:, kh : kh + 1, 0:128].bitcast(ui),
                        )
                if kop == 2:
                    # row-sum corrections from host-precomputed k-summed T:
                    # sa[o, i] = sum_k M[i, k, o] = (TS.T @ x.T)[o, i]
                    sa_ps = d2psum.tile([64, 256], fp32, tag="d2")
                    for cc in range(4):
                        nc.tensor.matmul(
                            sa_ps[:, :],
                            tsum[:, cc * 64 : (cc + 1) * 64],
                            xt[:, cc * 256 : (cc + 1) * 256],
                            start=(cc == 0),
                            stop=(cc == 3),
                        )
                    nc.scalar.copy(sa2[:, 0:256], sa_ps[:, :])
                    nc.scalar.copy(sa2[:, 256:512], sa_ps[:, :])
                if kop in stair:
                    c0, c1 = stair[kop]
                    mx_pre[0] = emit_tt(0, 0, c0, c1, mx_pre.get(0))
                if kop in pref_after:
                    for si in pref_after[kop]:
                        mx_pre[si] = emit_tt(si, 0)

            # odd-shift alignment copies, spread over the first 8 slots on
            # ACT: mtb[.., j] = mta[.., j+1]
            mtb_jobs = [(h, kh) for h in (0, 1) for kh in range(NH)]

            def emit_mtb(n):
                while n > 0 and mtb_jobs:
                    h, kh = mtb_jobs.pop(0)
                    ba = kh * CWA
                    nc.scalar.copy(
                        mtb[h][:, ba : ba + 383], mta[h][:, ba + 1 : ba + 384]
                    )
                    n -= 1

            # Phase 2, per shift slot s (t = t0 + s):
            #   2 DVE max instrs (fp16 2x, one per chunk-half)
            #   -> PE: 19 column-tiled matmul pairs (even half -> psum rows
            #   0:64, odd half -> 64:128), ACT copies /16 to fp16, PE fold
            #   matmul adds the halves + 2 row-sum corrections -> ACT
            #   exp(scale=-32, bias) -> PE accumulates e into self/pair.
            # Staggered PE pipeline: per slot, the queue gets
            #   lo-run(s) | fold+corr(s-1) | acc(s-2) | hi-run(s)
            # so no matmul ever heads the queue waiting on an ACT result
            # (dsb of s-1 and exp of s-2 are long done by the time their
            # consumers are reached), and the lo/hi runs on opposite array
            # column halves stay adjacent enough to overlap.
            def emit_fc(si):
                """fold + corrections for slot order[si] -> dp2, exp -> e."""
                s = order[si]
                dsb = state[si]["dsb"]
                dp2 = d2psum.tile([64, 256], fp32, tag="d2")
                nc.tensor.matmul(
                    dp2[:, :], ss[:, 0:64], dsb[:, :], start=True, stop=False
                )
                nc.tensor.matmul(
                    dp2[:, :], nh[:, :], sa2[:, 0:256], start=False, stop=False
                )
                nc.tensor.matmul(
                    dp2[:, :],
                    nh[:, :],
                    sa2[:, bass.ds(vp0 + s, 256)],
                    start=False,
                    stop=True,
                )
                e = epool.tile([64, 256], bf16, tag="e")
                nc.scalar.activation(
                    e[:, :],
                    dp2[:, :],
                    Act.Exp,
                    bias=bias[:, s : s + 1],
                    scale=-2.0 / DSB_SCALE,
                )
                state[si]["e"] = e

            def emit_ac(si):
                """accumulate e of slot order[si] into self/pair psums."""
                s = order[si]
                e = state[si]["e"]
                nc.tensor.matmul(
                    ps_self[:, :],
                    ident[:, :],
                    e[:, :],
                    start=(si == 0),
                    stop=(si == NSLOT - 1),
                )
                nc.tensor.matmul(
                    ps_pair[:, bass.ds(vp0 + s, 256)],
                    ident[:, :],
                    e[:, :],
                    start=False,
                    stop=(si == NSLOT - 1),
                    skip_group_check=True,
                )

            state = {}
            for si, s in enumerate(order):
                if si in mx_pre:
                    m3a = mx_pre.pop(si)
                else:
                    m3a = emit_tt(si, 0)
                if si == NSLOT - 1:
                    # split the last h1 max so its hi matmuls start earlier
                    m3b = emit_tt(si, 1, 0, 10)
                    emit_tt(si, 1, 10, NH, m3b)
                else:
                    m3b = emit_tt(si, 1)
                if si >= 1:
                    emit_fc(si - 1)
                if si >= 2:
                    emit_ac(si - 2)
                dp = dpsum.tile([128, 256], fp32, tag="dp")
                # interleaved column-tiled pairs: lo/hi matmuls adjacent in
                # the queue run concurrently in opposite array halves
                for c in range(NH):
                    nc.tensor.matmul(
                        dp[0:64, :],
                        ss[:, 0:64],
                        m3a[:, c, :],
                        start=(c == 0),
                        stop=(c == NH - 1),
                        tile_position=(0, 0),
                    )
                    nc.tensor.matmul(
                        dp[64:128, :],
                        ss[:, 0:64],
                        m3b[:, c, :],
                        start=(c == 0 and START_HI),
                        stop=(c == NH - 1),
                        tile_position=(0, 64),
                        skip_group_check=True,
                    )
                dsb = spool.tile([128, 256], fp16, tag="dsb")
                nc.scalar.mul(dsb[:, :], dp[:, :], DSB_SCALE)
                state[si] = {"dsb": dsb}
                if si < 8:
                    emit_mtb(5)
            emit_fc(NSLOT - 1)
            emit_ac(NSLOT - 2)
            emit_ac(NSLOT - 1)

            # raw accumulators out; the host combine (which already sums
            # the 8 cores) folds pair[0:256]+pair[256:512]+self
            outsb = cpool.tile([64, 768], fp32)
            nc.scalar.copy(outsb[:, 0:512], ps_pair[:, :])
            nc.scalar.copy(outsb[:, 512:768], ps_self[:, :])
            nc.sync.dma_start(out_d[:, :], outsb[:, :])

    nc.compile()
    return nc


def get_nc():
    if "nc" not in _NC_CACHE:
        _NC_CACHE["nc"] = _build_nc()
    return _NC_CACHE["nc"]


def host_inputs(x, T):
    """Host-side shard prep: returns the 8 per-core input maps."""
    x = np.asarray(x, dtype=np.float32)
    T = np.asarray(T, dtype=np.float32)
    T2p = np.zeros((F, KOP), np.float32)
    T2p[:, :KO] = T.reshape(F, KO)
    # tt[p, ko*512 + cc*128 + j] = T2p[cc*128+p, ko*128+j]
    tt = (
        np.ascontiguousarray(
            T2p.reshape(4, 128, NCH, 128).transpose(1, 2, 0, 3)
        )
        .reshape(128, NCH * 512)
        .astype(np.float16)
    )
    # xt[p, cc*256 + i] = x[i, cc*128+p]
    xt = (
        np.ascontiguousarray(x.T.reshape(4, 128, B).transpose(1, 0, 2))
        .reshape(128, 1024)
        .astype(np.float16)
    )
    # tsum[p, cc*64 + o] = TS[cc*128+p, o],  TS[f, o] = sum_k T[f, k, o]
    TS = T.reshape(F, K, O).sum(axis=1)
    tsum = (
        np.ascontiguousarray(TS.reshape(4, 128, O).transpose(1, 0, 2))
        .reshape(128, 256)
        .astype(np.float16)
    )
    ss = (np.arange(128)[:, None] % 64 == np.arange(64)[None, :]).astype(
        np.float16
    )
    import ml_dtypes
    nh = (-0.5 * DSB_SCALE * np.eye(64)).astype(np.float16)
    ident = np.eye(64).astype(ml_dtypes.bfloat16)
    in_maps = []
    for c in range(8):
        offs = np.array([[16 * c + 1, 0]], np.int32)
        biases = np.zeros((64, NSLOT), np.float32)
        if c == 7:
            biases[:, 15] = -LN2  # t = 128: every pair covered twice
        in_maps.append(
            {
                "xt": xt,
                "tt": tt,
                "tsum": tsum,
                "ssel": ss,
                "nhalf": nh,
                "ident": ident,
                "offs": offs,
                "bias": biases,
            }
        )
    return in_maps


def combine(results):
    """Sum per-core partial outputs [64,256] -> full [256,64] fp32.

    The reference computes sum_j exp(-d) (including the j=i term, = 1.0) in
    fp32 and then subtracts 1.0. Replicate those fp32 semantics exactly: the
    off-diagonal terms here are ~1e-25 and are fully absorbed by the +1.
    """
    acc = np.zeros((64, 256), np.float64)
    for r in results:
        o = r["out"].astype(np.float64)
        acc += o[:, 0:256] + o[:, 256:512] + o[:, 512:768]
    full = np.ascontiguousarray(acc.T).astype(np.float32)
    return (np.float32(1.0) + full) - np.float32(1.0)


def run_on_hw(x, T, trace=False):
    from concourse.bass_utils import run_bass_kernel_spmd

    nc = get_nc()
    in_maps = host_inputs(x, T)
    res = run_bass_kernel_spmd(
        nc, in_maps, core_ids=list(range(8)), trace=trace
    )
    return combine(res.results), res


def kernel(x, T):
    out, _ = run_on_hw(x, T, trace=False)
    return out
